# revision 22
# baseline (speedup 1.0000x reference)
"""Trainium2 Bass kernel for BaseLayerWithLoRA:
    y = x @ W^T + b + (x @ lora_A^T) @ lora_B^T
  x [4,2048,4096] f32, W [4096,4096], b [4096], lora_A [16,4096], lora_B [4096,16]

Sharding: token-parallel across 8 cores (1024 tokens each, full O per core).
No collectives needed; LoRA is computed per-core on its own token slice.

Per-core device program (all matmuls in float32r, full PE rate at N>=256):
  phase A: arT[r=16, 1024] = lora_A @ x_c^T        (64 matmuls)
  main:    outT[o-tile 128, tok 512] accumulated in PSUM over 32 K-chunks
           + 1 extra K=16 matmul adding lora_B^T-slice @ arT (packed into the
           same per-o-tile weight blob, so one DMA covers both)
           bias fused into the PSUM->SBUF eviction (DVE tensor_scalar_add
           with a per-partition scalar operand).
Host does data layout only (transposes / tiling / packing), no arithmetic.

Built on bacc.Bacc so compile() runs move_matmul_waits_to_ldweights and
generate_event_semaphores — TRN2 instructions hold only one sync wait, and
those passes spill excess waits onto EventSemaphore instructions.
"""

import sys

if "/opt/trn_rl_repo" not in sys.path:
    sys.path.insert(0, "/opt/trn_rl_repo")

import numpy as np

B, S, I, O, R = 4, 2048, 4096, 4096, 16
NCORES = 8
NTOK = B * S                 # 8192 tokens
TPC = NTOK // NCORES         # 1024 tokens per core


def build_nc(tpc=TPC, i_dim=I, o_dim=O, r=R, tok_tile=512):
    import concourse.bacc as bacc
    import concourse.mybir as mybir
    import concourse.tile as tile

    KC = i_dim // 128        # contraction chunks
    OT = o_dim // 128        # output-row tiles
    TT = tpc // tok_tile     # token tiles
    WF = KC * 128 + 128      # per-o-tile weight blob free size (W chunk + lora_B^T)
    f32 = mybir.dt.float32
    f32r = mybir.dt.float32r

    nc = bacc.Bacc("TRN2", target_bir_lowering=False, debug=False)
    xt = nc.declare_dram_parameter("xt", [KC, 128, tpc], f32r, isOutput=False)
    wt = nc.declare_dram_parameter("wt", [OT, 128, WF], f32r, isOutput=False)
    at = nc.declare_dram_parameter("at", [128, KC, r], f32r, isOutput=False)
    bias = nc.declare_dram_parameter("bias", [128, OT], f32, isOutput=False)
    out = nc.declare_dram_parameter("out", [OT, 128, tpc], f32, isOutput=True)

    with tile.TileContext(nc) as tc:
        with (
            tc.tile_pool(name="const", bufs=1) as constp,
            tc.tile_pool(name="xpool", bufs=KC) as xpool,
            tc.tile_pool(name="wpool", bufs=2) as wpool,
            tc.tile_pool(name="opool", bufs=3) as opool,
            tc.tile_pool(name="psum", bufs=4, space="PSUM") as psum_pool,
            tc.tile_pool(name="psum_ar", bufs=1, space="PSUM") as psum_ar_pool,
        ):
            at_sb = constp.tile([128, KC, r], f32r)
            nc.sync.dma_start(at_sb[:], at[:])
            b_sb = constp.tile([128, OT], f32)
            nc.sync.dma_start(b_sb[:], bias[:])

            # x^T resident in SBUF as one tile per 128-row K-chunk, so each
            # matmul depends on exactly one DMA.
            xts = []
            for kc in range(KC):
                x_t = xpool.tile([128, tpc], f32r, tag="xchunk", name=f"xchunk{kc}")
                nc.sync.dma_start(x_t[:], xt[kc])
                xts.append(x_t)

            # phase A: arT[r, tpc] = lora_A @ x_c^T  (chunk-outer so compute
            # starts as soon as each x chunk lands)
            arT_sb = constp.tile([r, tpc], f32r)
            pas = [
                psum_ar_pool.tile([r, tok_tile], f32, name=f"pa{h}")
                for h in range(TT)
            ]
            for kc in range(KC):
                for h in range(TT):
                    ts = slice(h * tok_tile, (h + 1) * tok_tile)
                    nc.tensor.matmul(
                        pas[h][:],
                        at_sb[:, kc, :],
                        xts[kc][:, ts],
                        start=(kc == 0),
                        stop=(kc == KC - 1),
                    )
            for h in range(TT):
                ts = slice(h * tok_tile, (h + 1) * tok_tile)
                nc.vector.tensor_copy(arT_sb[:, ts], pas[h][:])

            # main: per o-tile, one DMA brings the W chunk + lora_B^T slice;
            # 32 K-chunk matmuls + 1 lora matmul accumulate into one PSUM
            # bank; eviction adds the bias and DMAs out.
            for ot in range(OT):
                w_sb = wpool.tile([128, WF], f32r)
                nc.sync.dma_start(w_sb[:], wt[ot])
                for h in range(TT):
                    ts = slice(h * tok_tile, (h + 1) * tok_tile)
                    ps = psum_pool.tile([128, tok_tile], f32)
                    for kc in range(KC):
                        nc.tensor.matmul(
                            ps[:],
                            w_sb[:, kc * 128 : (kc + 1) * 128],
                            xts[kc][:, ts],
                            start=(kc == 0),
                            stop=False,
                        )
                    nc.tensor.matmul(
                        ps[:],
                        w_sb[0:r, KC * 128 : KC * 128 + 128],
                        arT_sb[:, ts],
                        start=False,
                        stop=True,
                    )
                    o_sb = opool.tile([128, tok_tile], f32)
                    nc.vector.tensor_scalar_add(
                        o_sb[:], ps[:], b_sb[:, ot : ot + 1]
                    )
                    nc.sync.dma_start(out[ot, :, ts], o_sb[:])
    nc.compile()
    return nc


def prep_inputs(x, W, b, lora_A, lora_B, tpc=TPC, ncores=NCORES):
    """Host-side layout marshalling (no arithmetic). Returns per-core input maps."""
    i_dim, o_dim, r = W.shape[1], W.shape[0], lora_A.shape[0]
    ntok = tpc * ncores
    x = np.ascontiguousarray(x, dtype=np.float32).reshape(ntok, i_dim)
    W = np.ascontiguousarray(W, dtype=np.float32)
    b = np.ascontiguousarray(b, dtype=np.float32)
    lora_A = np.ascontiguousarray(lora_A, dtype=np.float32)
    lora_B = np.ascontiguousarray(lora_B, dtype=np.float32)

    KC, OT = i_dim // 128, o_dim // 128
    WF = KC * 128 + 128
    # wt blob per o-tile: [ki, kc*128+oo] = W[ot*128+oo, kc*128+ki],
    # last 128 cols rows 0:r = lora_B^T slice: [rr, oo] = lora_B[ot*128+oo, rr]
    wtb = np.zeros((OT, 128, WF), dtype=np.float32)
    wtb[:, :, : KC * 128] = (
        W.reshape(OT, 128, KC, 128).transpose(0, 3, 2, 1).reshape(OT, 128, KC * 128)
    )
    wtb[:, :r, KC * 128 :] = lora_B.reshape(OT, 128, r).transpose(0, 2, 1)
    # at[ki, kc, r] = lora_A[r, kc*128+ki]
    at = np.ascontiguousarray(lora_A.T.reshape(KC, 128, r).transpose(1, 0, 2))
    # bias[p, ot] = b[ot*128+p]
    bias = np.ascontiguousarray(b.reshape(OT, 128).T)

    in_maps = []
    for c in range(ncores):
        xc = x[c * tpc : (c + 1) * tpc]  # [tpc, i_dim]
        # xt[kc, ki, t] = xc[t, kc*128+ki]
        xtc = np.ascontiguousarray(xc.reshape(tpc, KC, 128).transpose(1, 2, 0))
        in_maps.append({"xt": xtc, "wt": wtb, "at": at, "bias": bias})
    return in_maps


def assemble_output(results):
    # each core: out[OT, 128, tpc] == y_c^T; tokens are block-sharded
    outT = np.concatenate([r["out"] for r in results], axis=2)  # [OT,128,ntok]
    o_dim = outT.shape[0] * 128
    ntok = outT.shape[2]
    y = outT.reshape(o_dim, ntok).T  # [ntok, o_dim]
    return np.ascontiguousarray(y)


def run(trace=False, trace_kwargs=None, **inputs):
    from concourse.bass_utils import run_bass_kernel_spmd

    nc = build_nc()
    in_maps = prep_inputs(**inputs)
    res = run_bass_kernel_spmd(
        nc,
        in_maps,
        list(range(NCORES)),
        trace=trace,
        trace_kwargs=trace_kwargs or {},
    )
    return assemble_output(res.results).reshape(B, S, O), res


def kernel(**inputs):
    y, _ = run(trace=False, **inputs)
    return y


# revision 23
# speedup vs baseline: 1.1151x; 1.1151x over previous
"""Trainium2 Bass kernel for BaseLayerWithLoRA:
    y = x @ W^T + b + (x @ lora_A^T) @ lora_B^T
  x [4,2048,4096] f32, W [4096,4096], b [4096], lora_A [16,4096], lora_B [4096,16]

Sharding: token-parallel across 8 cores (1024 tokens each, full O per core).
No collectives needed; LoRA is computed per-core on its own token slice.

Per-core device program (all matmuls in float32r, full PE rate at N>=256):
  phase A: arT[r=16, 1024] = lora_A @ x_c^T        (64 matmuls)
  main:    outT[o-tile 128, tok 512] accumulated in PSUM over 32 K-chunks
           + 1 extra K=16 matmul adding lora_B^T-slice @ arT (packed into the
           same per-o-tile weight blob, so one DMA covers both)
           bias fused into the PSUM->SBUF eviction (DVE tensor_scalar_add
           with a per-partition scalar operand).
Host does data layout only (transposes / tiling / packing), no arithmetic.

Built on bacc.Bacc so compile() runs move_matmul_waits_to_ldweights and
generate_event_semaphores — TRN2 instructions hold only one sync wait, and
those passes spill excess waits onto EventSemaphore instructions.
"""

import sys

if "/opt/trn_rl_repo" not in sys.path:
    sys.path.insert(0, "/opt/trn_rl_repo")

import numpy as np

B, S, I, O, R = 4, 2048, 4096, 4096, 16
NCORES = 8
NTOK = B * S                 # 8192 tokens
TPC = NTOK // NCORES         # 1024 tokens per core


def build_nc(tpc=TPC, i_dim=I, o_dim=O, r=R, tok_tile=512, mm_dtype="bfloat16"):
    import concourse.bacc as bacc
    import concourse.mybir as mybir
    import concourse.tile as tile

    KC = i_dim // 128        # contraction chunks
    OT = o_dim // 128        # output-row tiles
    TT = tpc // tok_tile     # token tiles
    WF = KC * 128 + 128      # per-o-tile weight blob free size (W chunk + lora_B^T)
    f32 = mybir.dt.float32
    f32r = getattr(mybir.dt, mm_dtype)

    nc = bacc.Bacc("TRN2", target_bir_lowering=False, debug=False)
    xt = nc.declare_dram_parameter("xt", [KC, 128, tpc], f32r, isOutput=False)
    wt = nc.declare_dram_parameter("wt", [OT, 128, WF], f32r, isOutput=False)
    at = nc.declare_dram_parameter("at", [128, KC, r], f32r, isOutput=False)
    bias = nc.declare_dram_parameter("bias", [128, OT], f32, isOutput=False)
    out = nc.declare_dram_parameter("out", [OT, 128, tpc], f32, isOutput=True)

    with tile.TileContext(nc) as tc:
        with (
            tc.tile_pool(name="const", bufs=1) as constp,
            tc.tile_pool(name="xpool", bufs=KC) as xpool,
            tc.tile_pool(name="wpool", bufs=3) as wpool,
            tc.tile_pool(name="opool", bufs=3) as opool,
            tc.tile_pool(name="psum", bufs=6, space="PSUM") as psum_pool,
            tc.tile_pool(name="psum_ar", bufs=1, space="PSUM") as psum_ar_pool,
        ):
            at_sb = constp.tile([128, KC, r], f32r)
            nc.sync.dma_start(at_sb[:], at[:])
            b_sb = constp.tile([128, OT], f32)
            nc.sync.dma_start(b_sb[:], bias[:])

            # x^T resident in SBUF as one tile per 128-row K-chunk, so each
            # matmul depends on exactly one DMA.
            xts = []
            for kc in range(KC):
                x_t = xpool.tile([128, tpc], f32r, tag="xchunk", name=f"xchunk{kc}")
                nc.sync.dma_start(x_t[:], xt[kc])
                xts.append(x_t)

            # phase A: arT[r, tpc] = lora_A @ x_c^T  (chunk-outer so compute
            # starts as soon as each x chunk lands)
            arT_sb = constp.tile([r, tpc], f32r)
            pas = [
                psum_ar_pool.tile([r, tok_tile], f32, name=f"pa{h}")
                for h in range(TT)
            ]
            for kc in range(KC):
                for h in range(TT):
                    ts = slice(h * tok_tile, (h + 1) * tok_tile)
                    nc.tensor.matmul(
                        pas[h][:],
                        at_sb[:, kc, :],
                        xts[kc][:, ts],
                        start=(kc == 0),
                        stop=(kc == KC - 1),
                    )
            for h in range(TT):
                ts = slice(h * tok_tile, (h + 1) * tok_tile)
                nc.vector.tensor_copy(arT_sb[:, ts], pas[h][:])

            # main: per o-tile, one DMA brings the W chunk + lora_B^T slice;
            # 32 K-chunk matmuls + 1 lora matmul accumulate into one PSUM
            # bank; eviction adds the bias and DMAs out.
            for ot in range(OT):
                w_sb = wpool.tile([128, WF], f32r)
                nc.sync.dma_start(w_sb[:], wt[ot])
                for h in range(TT):
                    ts = slice(h * tok_tile, (h + 1) * tok_tile)
                    ps = psum_pool.tile([128, tok_tile], f32)
                    for kc in range(KC):
                        nc.tensor.matmul(
                            ps[:],
                            w_sb[:, kc * 128 : (kc + 1) * 128],
                            xts[kc][:, ts],
                            start=(kc == 0),
                            stop=False,
                        )
                    nc.tensor.matmul(
                        ps[:],
                        w_sb[0:r, KC * 128 : KC * 128 + 128],
                        arT_sb[:, ts],
                        start=False,
                        stop=True,
                    )
                    o_sb = opool.tile([128, tok_tile], f32)
                    nc.vector.tensor_scalar_add(
                        o_sb[:], ps[:], b_sb[:, ot : ot + 1]
                    )
                    nc.sync.dma_start(out[ot, :, ts], o_sb[:])
    nc.compile()
    return nc


def prep_inputs(x, W, b, lora_A, lora_B, tpc=TPC, ncores=NCORES,
                mm_dtype="bfloat16"):
    """Host-side layout marshalling (layout + dtype cast only)."""
    import ml_dtypes

    np_mm = np.float32 if mm_dtype == "float32r" else np.dtype(ml_dtypes.bfloat16)
    i_dim, o_dim, r = W.shape[1], W.shape[0], lora_A.shape[0]
    ntok = tpc * ncores
    x = np.ascontiguousarray(x, dtype=np.float32).reshape(ntok, i_dim)
    W = np.ascontiguousarray(W, dtype=np.float32)
    b = np.ascontiguousarray(b, dtype=np.float32)
    lora_A = np.ascontiguousarray(lora_A, dtype=np.float32)
    lora_B = np.ascontiguousarray(lora_B, dtype=np.float32)

    KC, OT = i_dim // 128, o_dim // 128
    WF = KC * 128 + 128
    # wt blob per o-tile: [ki, kc*128+oo] = W[ot*128+oo, kc*128+ki],
    # last 128 cols rows 0:r = lora_B^T slice: [rr, oo] = lora_B[ot*128+oo, rr]
    wtb = np.zeros((OT, 128, WF), dtype=np_mm)
    wtb[:, :, : KC * 128] = (
        W.reshape(OT, 128, KC, 128).transpose(0, 3, 2, 1).reshape(OT, 128, KC * 128)
    ).astype(np_mm)
    wtb[:, :r, KC * 128 :] = (
        lora_B.reshape(OT, 128, r).transpose(0, 2, 1).astype(np_mm)
    )
    # at[ki, kc, r] = lora_A[r, kc*128+ki]
    at = np.ascontiguousarray(
        lora_A.T.reshape(KC, 128, r).transpose(1, 0, 2).astype(np_mm)
    )
    # bias[p, ot] = b[ot*128+p]
    bias = np.ascontiguousarray(b.reshape(OT, 128).T)

    in_maps = []
    for c in range(ncores):
        xc = x[c * tpc : (c + 1) * tpc]  # [tpc, i_dim]
        # xt[kc, ki, t] = xc[t, kc*128+ki]
        xtc = np.ascontiguousarray(
            xc.reshape(tpc, KC, 128).transpose(1, 2, 0).astype(np_mm)
        )
        in_maps.append({"xt": xtc, "wt": wtb, "at": at, "bias": bias})
    return in_maps


def assemble_output(results):
    # each core: out[OT, 128, tpc] == y_c^T; tokens are block-sharded
    outT = np.concatenate([r["out"] for r in results], axis=2)  # [OT,128,ntok]
    o_dim = outT.shape[0] * 128
    ntok = outT.shape[2]
    y = outT.reshape(o_dim, ntok).T  # [ntok, o_dim]
    return np.ascontiguousarray(y)


def run(trace=False, trace_kwargs=None, mm_dtype="bfloat16", **inputs):
    from concourse.bass_utils import run_bass_kernel_spmd

    nc = build_nc(mm_dtype=mm_dtype)
    in_maps = prep_inputs(mm_dtype=mm_dtype, **inputs)
    res = run_bass_kernel_spmd(
        nc,
        in_maps,
        list(range(NCORES)),
        trace=trace,
        trace_kwargs=trace_kwargs or {},
    )
    return assemble_output(res.results).reshape(B, S, O), res


def kernel(**inputs):
    y, _ = run(trace=False, **inputs)
    return y
